# revision 33
# baseline (speedup 1.0000x reference)
"""LCNN conv2d kernel for Trainium2 (8 NeuronCores, batch-sharded).

Math: out[b,o,h,w] = sum_d Wmat[o,d] * conv2d(x, dictionary)[b,d,h,w]
where Wmat is the scatter-add of lookup_coefficients into [O, D].

Device strategy (per core, 2 batches), all-bf16:
 - stage 1: conv with the D=100 (padded to 128) dictionary as 5 accumulating
   K=128 matmuls per output tile: 3 kw-pair matmuls on XXa=[x ; x+1col],
   1 kh-pair matmul (kh1/kh2 @ kw2) on XXc=[x+1row ; x+2row], and 1
   single-tap matmul (kh0 @ kw2, zero-padded K) on XXa.
 - stage 2: [O=256, D] channel-mix as 2 matmuls (128-wide halves) on the
   bf16 copy of the conv PSUM tile.
 - output staged to SBUF as bf16 and DMA'd at half the f32 byte cost;
   host upcasts to f32.
"""
import os
import sys

for _p in ("/opt/trn_rl_repo", "/root/.axon_site/_ro/trn_rl_repo"):
    if os.path.isdir(_p) and _p not in sys.path:
        sys.path.insert(0, _p)

import ml_dtypes
import numpy as np
from contextlib import ExitStack

from concourse import bacc, mybir, tile
from concourse.bass_utils import run_bass_kernel_spmd

# problem shapes (hardcoded per contract)
B, CIN, H, W = 16, 64, 96, 96
D, O = 100, 256
DP = 128                   # D padded to full PE width (enables FWL)
NCORES = 8
BPC = B // NCORES          # batches per core
PH, PW = H + 2, W + 2      # zero-padded spatial
F = BPC * PH * PW          # per-partition x extent
R = 4                      # output rows per matmul tile
NT = H // R                # h-tiles per batch
N = R * W                  # matmul free size (384)
LAG = 3                    # stage-2 pipeline lag (tiles)
PB = 512                   # psum bank stride (f32 elems)
bf16 = mybir.dt.bfloat16
f32 = mybir.dt.float32

_NC_CACHE = {}


def _build():
    nc = bacc.Bacc(None, target_bir_lowering=False, debug=False)
    xp = nc.declare_dram_parameter("xp", [128, F], bf16, isOutput=False)
    xc = nc.declare_dram_parameter("xc", [128, F], bf16, isOutput=False)
    wst = nc.declare_dram_parameter("wst", [128, 6 * DP], bf16, isOutput=False)
    wm = nc.declare_dram_parameter("wm", [DP, O], bf16, isOutput=False)
    out = nc.declare_dram_parameter("out", [BPC, O, H, W], bf16, isOutput=True)

    with tile.TileContext(nc) as tc, ExitStack() as ctx:
        sb = ctx.enter_context(tc.tile_pool(name="sb", bufs=1))
        c1p = ctx.enter_context(tc.tile_pool(name="c1p", bufs=5))
        stgp = ctx.enter_context(tc.tile_pool(name="stgp", bufs=12))
        pcp = ctx.enter_context(tc.tile_pool(name="pcp", bufs=2, space="PSUM"))
        pop = ctx.enter_context(tc.tile_pool(name="pop", bufs=2, space="PSUM"))

        XXa = sb.tile([128, F], bf16)
        XXc = sb.tile([128, F], bf16)
        wst_s = sb.tile([128, 6 * DP], bf16)
        wm_s = sb.tile([DP, O], bf16)
        # weights via gpsimd SWDGE so the sync ring's head slot goes to the
        # first x chunks; both land well before the first real matmul
        nc.gpsimd.dma_start(wst_s[:], wst[:])
        nc.gpsimd.dma_start(wm_s[:], wm[:])

        # x chunk loads, interleaved XXa/XXc front-to-back on the sync HWDGE
        # ring; small leading chunks minimize time-to-first-matmul.
        rows = [7, 7] + [14] * 13
        a = 0
        for nr in rows:
            L = nr * PW
            nc.sync.dma_start(XXa[:, a:a + L], xp[:, a:a + L])
            nc.sync.dma_start(XXc[:, a:a + L], xc[:, a:a + L])
            a += L
        assert a == F

        XAv = XXa.rearrange("p (b h w) -> p b h w", b=BPC, h=PH, w=PW)
        XCv = XXc.rearrange("p (b h w) -> p b h w", b=BPC, h=PH, w=PW)

        # PE warm-up: dummy matmuls on a zeroed SBUF tile run while the x
        # chunks stream in, so HAM reaches K=8/8 before the first real matmul
        # and the ramp penalty is paid during otherwise-idle time. They write
        # the first pair's conv PSUM tile, which the real group resets.
        warm = sb.tile([128, 512], bf16)
        nc.vector.memset(warm[:], 0)
        wq = pcp.tile([128, 2 * PB], f32, name="pcq")
        for _ in range(12):
            nc.tensor.matmul(wq[:, 0:512], warm[:, 0:128], warm[:],
                             start=True, stop=True, skip_group_check=True)
        state = {"warmq": wq}

        def stage1_pair(b, t0):
            """Two tiles' conv groups; their half-empty (kh0,kw2) matmuls are
            emitted adjacently as K=64 row-group tiles (rows 0:64 / 64:128)
            so they execute concurrently in the PE array."""
            t1 = t0 + 1
            h0, h1 = t0 * R, t1 * R
            pcq = state.pop("warmq", None)
            if pcq is None:
                pcq = pcp.tile([128, 2 * PB], f32, name="pcq")
            pcqv = pcq.rearrange("p (u n) -> p u n", u=2)
            pc0 = pcqv[:, 0, 0:N]
            pc1 = pcqv[:, 1, 0:N]
            nc.tensor.matmul(
                pc0, wst_s[0:64, 4 * DP:5 * DP],
                XAv[0:64, b, h0:h0 + R, 2:PW], start=True, stop=False)
            nc.tensor.matmul(
                pc1, wst_s[64:128, 5 * DP:6 * DP],
                XCv[64:128, b, h1 - 2:h1 + 2, 2:PW], start=True, stop=False)
            for t, pc, hh in ((t0, pc0, h0), (t1, pc1, h1)):
                # kw-pairs (kw0,kw1) for each kh on XXa
                for kh in range(3):
                    nc.tensor.matmul(
                        pc, wst_s[:, kh * DP:(kh + 1) * DP],
                        XAv[:, b, hh + kh:hh + kh + R, 0:W],
                        start=False, stop=False)
                # kh-pair (kh1,kh2) @ kw2 on XXc (row-shifted banks)
                nc.tensor.matmul(
                    pc, wst_s[:, 3 * DP:4 * DP],
                    XCv[:, b, hh:hh + R, 2:PW],
                    start=False, stop=True)
            # one strided copy evacuates both tiles' conv PSUM banks; the
            # final pair splits across both engines to shorten the drain
            c1q = c1p.tile([128, 2 * N], bf16, name="c1q")
            if b == BPC - 1 and t1 == NT - 1:
                nc.vector.tensor_copy(c1q[:, 0:N], pcqv[:, 0, 0:N])
                nc.scalar.copy(c1q[:, N:2 * N], pcqv[:, 1, 0:N])
            elif (t0 // 2) % 2 == 0:
                nc.vector.tensor_copy(
                    c1q.rearrange("p (u n) -> p u n", u=2), pcqv[:, :, 0:N])
            else:
                nc.scalar.copy(
                    c1q.rearrange("p (u n) -> p u n", u=2), pcqv[:, :, 0:N])
            state[(b, t0)] = c1q[:, 0:N]
            state[(b, t1)] = c1q[:, N:2 * N]

        def stage2(b, t):
            c1 = state.pop((b, t))
            po = pop.tile([128, 2 * PB], f32, name="po")
            pov = po.rearrange("p (u n) -> p u n", u=2)
            nc.tensor.matmul(pov[:, 0, 0:N], wm_s[:, 0:128], c1,
                             start=True, stop=True)
            nc.tensor.matmul(pov[:, 1, 0:N], wm_s[:, 128:256], c1,
                             start=True, stop=True)
            stg = stgp.tile([128, 2 * N], bf16, name="stg")
            stgv = stg.rearrange("p (u n) -> p u n", u=2)
            # single strided copy evacuates both 128-channel halves; opposite
            # engine from the same emission iteration's c1 copy. Final tiles
            # split across engines to shorten the drain.
            if b == BPC - 1 and t >= NT - 4:
                nc.scalar.copy(stgv[:, 0, :], pov[:, 0, 0:N])
                nc.vector.tensor_copy(stgv[:, 1, :], pov[:, 1, 0:N])
            elif t % 2 == 0:
                nc.scalar.copy(stgv[:], pov[:, :, 0:N])
            else:
                nc.vector.tensor_copy(stgv[:], pov[:, :, 0:N])
            # per-tile DMA: partition o carries channels {o, 128+o}, each a
            # contiguous 768 B run
            dst = out[b].rearrange("(u o) h w -> o u (h w)", u=2)[
                :, :, t * N:(t + 1) * N]
            # batch 1 outputs go out on the sync HWDGE ring, which is idle
            # once the input chunks have drained (b=1 stage2 starts ~then);
            # cheaper issue + shorter end-of-kernel drain than gpsimd SWDGE.
            if b == 0 or t == NT - 1:
                nc.gpsimd.dma_start(dst, stgv)
            else:
                nc.sync.dma_start(dst, stgv)

        NP = NT // 2
        PLAG = 2    # pairs of lag -> stage2 trails by 2*PLAG tiles
        for b in range(BPC):
            for p in range(NP):
                stage1_pair(b, 2 * p)
                if p >= PLAG:
                    stage2(b, 2 * (p - PLAG))
                    stage2(b, 2 * (p - PLAG) + 1)
            for t in range(NT - 2 * PLAG, NT):
                stage2(b, t)

    nc.compile()
    return nc


def _get_nc():
    if "nc" not in _NC_CACHE:
        _NC_CACHE["nc"] = _build()
    return _NC_CACHE["nc"]


def _prep_inputs(x, dictionary, lookup_coefficients, lookup_indices):
    x = np.asarray(x, dtype=np.float32)
    dic = np.asarray(dictionary, dtype=np.float32)
    coeff = np.asarray(lookup_coefficients, dtype=np.float32).reshape(O, -1)
    idx = np.asarray(lookup_indices).astype(np.int64).reshape(O, -1)

    wmat = np.zeros((O, D), np.float32)
    np.add.at(wmat, (np.arange(O)[:, None], idx), coeff)
    wmp = np.zeros((DP, O), np.float32)
    wmp[:D] = wmat.T
    wmp = wmp.astype(ml_dtypes.bfloat16)

    # stationary slabs [128, 5*DP]: 3 kw-pairs, the (kh1,kh2)@kw2 pair,
    # and the lone (kh0,kw2) tap (upper K rows zero).
    dt_ = dic.transpose(1, 0, 2, 3)                       # [cin, d, kh, kw]
    wstk = np.zeros((128, 6 * DP), np.float32)
    for kh in range(3):
        wstk[0:64, kh * DP:kh * DP + D] = dt_[:, :, kh, 0]
        wstk[64:128, kh * DP:kh * DP + D] = dt_[:, :, kh, 1]
    wstk[0:64, 3 * DP:3 * DP + D] = dt_[:, :, 1, 2]
    wstk[64:128, 3 * DP:3 * DP + D] = dt_[:, :, 2, 2]
    wstk[0:64, 4 * DP:4 * DP + D] = dt_[:, :, 0, 2]      # (kh0,kw2) rows 0:64
    wstk[64:128, 5 * DP:5 * DP + D] = dt_[:, :, 0, 2]    # (kh0,kw2) rows 64:128
    wstk = wstk.astype(ml_dtypes.bfloat16)

    xpad = np.zeros((B, CIN, PH, PW), np.float32)
    xpad[:, :, 1:H + 1, 1:W + 1] = x
    xpad = xpad.astype(ml_dtypes.bfloat16)

    in_maps = []
    for c in range(NCORES):
        xf = xpad[c * BPC:(c + 1) * BPC].transpose(1, 0, 2, 3).reshape(CIN, F)
        xpk = np.zeros((128, F), ml_dtypes.bfloat16)
        xck = np.zeros((128, F), ml_dtypes.bfloat16)
        xpk[0:64] = xf
        xpk[64:128, 0:F - 1] = xf[:, 1:]
        xck[0:64, 0:F - PW] = xf[:, PW:]
        xck[64:128, 0:F - 2 * PW] = xf[:, 2 * PW:]
        in_maps.append({
            "xp": np.ascontiguousarray(xpk),
            "xc": np.ascontiguousarray(xck),
            "wst": wstk, "wm": wmp,
        })
    return in_maps


def _run(in_maps, trace=False, **kw):
    nc = _get_nc()
    return run_bass_kernel_spmd(nc, in_maps, core_ids=list(range(NCORES)),
                                trace=trace, **kw)


def kernel(x, dictionary, lookup_coefficients, lookup_indices):
    in_maps = _prep_inputs(x, dictionary, lookup_coefficients, lookup_indices)
    res = _run(in_maps)
    outs = [np.asarray(res.results[c]["out"]).astype(np.float32)
            for c in range(NCORES)]
    return np.concatenate(outs, axis=0)


# revision 36
# speedup vs baseline: 1.0276x; 1.0276x over previous
"""LCNN conv2d kernel for Trainium2 (8 NeuronCores, batch-sharded).

Math: out[b,o,h,w] = sum_d Wmat[o,d] * conv2d(x, dictionary)[b,d,h,w]
where Wmat is the scatter-add of lookup_coefficients into [O, D].

Device strategy (per core, 2 batches), all-bf16:
 - stage 1: conv with the D=100 (padded to 128) dictionary as 5 accumulating
   K=128 matmuls per output tile: 3 kw-pair matmuls on XXa=[x ; x+1col],
   1 kh-pair matmul (kh1/kh2 @ kw2) on XXc=[x+1row ; x+2row], and 1
   single-tap matmul (kh0 @ kw2, zero-padded K) on XXa.
 - stage 2: [O=256, D] channel-mix as 2 matmuls (128-wide halves) on the
   bf16 copy of the conv PSUM tile.
 - output staged to SBUF as bf16 and DMA'd at half the f32 byte cost;
   host upcasts to f32.
"""
import os
import sys

for _p in ("/opt/trn_rl_repo", "/root/.axon_site/_ro/trn_rl_repo"):
    if os.path.isdir(_p) and _p not in sys.path:
        sys.path.insert(0, _p)

import ml_dtypes
import numpy as np
from contextlib import ExitStack

from concourse import bacc, mybir, tile
from concourse.bass_utils import run_bass_kernel_spmd

# problem shapes (hardcoded per contract)
B, CIN, H, W = 16, 64, 96, 96
D, O = 100, 256
DP = 128                   # D padded to full PE width (enables FWL)
NCORES = 8
BPC = B // NCORES          # batches per core
PH, PW = H + 2, W + 2      # zero-padded spatial
F = BPC * PH * PW          # per-partition x extent
R = 4                      # output rows per matmul tile
NT = H // R                # h-tiles per batch
N = R * W                  # matmul free size (384)
LAG = 3                    # stage-2 pipeline lag (tiles)
PB = 512                   # psum bank stride (f32 elems)
bf16 = mybir.dt.bfloat16
f32 = mybir.dt.float32

_NC_CACHE = {}


def _build():
    nc = bacc.Bacc(None, target_bir_lowering=False, debug=False)
    xp = nc.declare_dram_parameter("xp", [128, F], bf16, isOutput=False)
    xc = nc.declare_dram_parameter("xc", [128, F], bf16, isOutput=False)
    wst = nc.declare_dram_parameter("wst", [128, 6 * DP], bf16, isOutput=False)
    wm = nc.declare_dram_parameter("wm", [DP, O], bf16, isOutput=False)
    out = nc.declare_dram_parameter("out", [BPC, O, H, W], bf16, isOutput=True)

    with tile.TileContext(nc) as tc, ExitStack() as ctx:
        sb = ctx.enter_context(tc.tile_pool(name="sb", bufs=1))
        c1p = ctx.enter_context(tc.tile_pool(name="c1p", bufs=5))
        stgp = ctx.enter_context(tc.tile_pool(name="stgp", bufs=12))
        pcp = ctx.enter_context(tc.tile_pool(name="pcp", bufs=2, space="PSUM"))
        pop = ctx.enter_context(tc.tile_pool(name="pop", bufs=2, space="PSUM"))

        XXa = sb.tile([128, F], bf16)
        XXc = sb.tile([128, F], bf16)
        wst_s = sb.tile([128, 6 * DP], bf16)
        wm_s = sb.tile([DP, O], bf16)
        # weights via gpsimd SWDGE so the sync ring's head slot goes to the
        # first x chunks; both land well before the first real matmul
        nc.gpsimd.dma_start(wst_s[:], wst[:])
        nc.gpsimd.dma_start(wm_s[:], wm[:])

        # x chunk loads, interleaved XXa/XXc front-to-back on the sync HWDGE
        # ring; small leading chunks minimize time-to-first-matmul.
        rows = [7, 7] + [14] * 13
        a = 0
        for nr in rows:
            L = nr * PW
            nc.sync.dma_start(XXa[:, a:a + L], xp[:, a:a + L])
            nc.sync.dma_start(XXc[:, a:a + L], xc[:, a:a + L])
            a += L
        assert a == F

        XAv = XXa.rearrange("p (b h w) -> p b h w", b=BPC, h=PH, w=PW)
        XCv = XXc.rearrange("p (b h w) -> p b h w", b=BPC, h=PH, w=PW)

        # PE warm-up: dummy matmuls on a zeroed SBUF tile run while the x
        # chunks stream in, so HAM reaches K=8/8 before the first real matmul
        # and the ramp penalty is paid during otherwise-idle time. They write
        # the first pair's conv PSUM tile, which the real group resets.
        warm = sb.tile([128, 512], bf16)
        nc.vector.memset(warm[:], 0)
        wq = pcp.tile([128, 2 * PB], f32, name="pcq")
        for _ in range(12):
            nc.tensor.matmul(wq[:, 0:512], warm[:, 0:128], warm[:],
                             start=True, stop=True, skip_group_check=True)
        state = {"warmq": wq}

        def stage1_pair(b, t0):
            """Two tiles' conv groups; their half-empty (kh0,kw2) matmuls are
            emitted adjacently as K=64 row-group tiles (rows 0:64 / 64:128)
            so they execute concurrently in the PE array."""
            t1 = t0 + 1
            h0, h1 = t0 * R, t1 * R
            pcq = state.pop("warmq", None)
            if pcq is None:
                pcq = pcp.tile([128, 2 * PB], f32, name="pcq")
            pcqv = pcq.rearrange("p (u n) -> p u n", u=2)
            pc0 = pcqv[:, 0, 0:N]
            pc1 = pcqv[:, 1, 0:N]
            nc.tensor.matmul(
                pc0, wst_s[0:64, 4 * DP:5 * DP],
                XAv[0:64, b, h0:h0 + R, 2:PW], start=True, stop=False)
            nc.tensor.matmul(
                pc1, wst_s[64:128, 5 * DP:6 * DP],
                XCv[64:128, b, h1 - 2:h1 + 2, 2:PW], start=True, stop=False)
            for t, pc, hh in ((t0, pc0, h0), (t1, pc1, h1)):
                # kw-pairs (kw0,kw1) for each kh on XXa
                for kh in range(3):
                    nc.tensor.matmul(
                        pc, wst_s[:, kh * DP:(kh + 1) * DP],
                        XAv[:, b, hh + kh:hh + kh + R, 0:W],
                        start=False, stop=False)
                # kh-pair (kh1,kh2) @ kw2 on XXc (row-shifted banks)
                nc.tensor.matmul(
                    pc, wst_s[:, 3 * DP:4 * DP],
                    XCv[:, b, hh:hh + R, 2:PW],
                    start=False, stop=True)
            # one strided copy evacuates both tiles' conv PSUM banks; the
            # final pair splits across both engines to shorten the drain
            c1q = c1p.tile([128, 2 * N], bf16, name="c1q")
            if (t0 // 2) % 2 == 0:
                nc.vector.tensor_copy(
                    c1q.rearrange("p (u n) -> p u n", u=2), pcqv[:, :, 0:N])
            else:
                nc.scalar.copy(
                    c1q.rearrange("p (u n) -> p u n", u=2), pcqv[:, :, 0:N])
            state[(b, t0)] = c1q[:, 0:N]
            state[(b, t1)] = c1q[:, N:2 * N]

        def stage2(b, t):
            c1 = state.pop((b, t))
            po = pop.tile([128, 2 * PB], f32, name="po")
            pov = po.rearrange("p (u n) -> p u n", u=2)
            nc.tensor.matmul(pov[:, 0, 0:N], wm_s[:, 0:128], c1,
                             start=True, stop=True)
            nc.tensor.matmul(pov[:, 1, 0:N], wm_s[:, 128:256], c1,
                             start=True, stop=True)
            stg = stgp.tile([128, 2 * N], bf16, name="stg")
            stgv = stg.rearrange("p (u n) -> p u n", u=2)
            # single strided copy evacuates both 128-channel halves; opposite
            # engine from the same emission iteration's c1 copy
            if t % 2 == 0:
                nc.scalar.copy(stgv[:], pov[:, :, 0:N])
            else:
                nc.vector.tensor_copy(stgv[:], pov[:, :, 0:N])
            # per-tile DMA: partition o carries channels {o, 128+o}, each a
            # contiguous 768 B run
            dst = out[b].rearrange("(u o) h w -> o u (h w)", u=2)[
                :, :, t * N:(t + 1) * N]
            # batch 1 outputs go out on the sync HWDGE ring, which is idle
            # once the input chunks have drained (b=1 stage2 starts ~then);
            # cheaper issue + shorter end-of-kernel drain than gpsimd SWDGE.
            if b == 0:
                nc.gpsimd.dma_start(dst, stgv)
            else:
                nc.sync.dma_start(dst, stgv)

        NP = NT // 2
        PLAG = 2    # pairs of lag -> stage2 trails by 2*PLAG tiles
        for b in range(BPC):
            for p in range(NP):
                stage1_pair(b, 2 * p)
                if p >= PLAG:
                    stage2(b, 2 * (p - PLAG))
                    stage2(b, 2 * (p - PLAG) + 1)
            for t in range(NT - 2 * PLAG, NT):
                stage2(b, t)

    nc.compile()
    return nc


def _get_nc():
    if "nc" not in _NC_CACHE:
        _NC_CACHE["nc"] = _build()
    return _NC_CACHE["nc"]


def _prep_inputs(x, dictionary, lookup_coefficients, lookup_indices):
    x = np.asarray(x, dtype=np.float32)
    dic = np.asarray(dictionary, dtype=np.float32)
    coeff = np.asarray(lookup_coefficients, dtype=np.float32).reshape(O, -1)
    idx = np.asarray(lookup_indices).astype(np.int64).reshape(O, -1)

    wmat = np.zeros((O, D), np.float32)
    np.add.at(wmat, (np.arange(O)[:, None], idx), coeff)
    wmp = np.zeros((DP, O), np.float32)
    wmp[:D] = wmat.T
    wmp = wmp.astype(ml_dtypes.bfloat16)

    # stationary slabs [128, 5*DP]: 3 kw-pairs, the (kh1,kh2)@kw2 pair,
    # and the lone (kh0,kw2) tap (upper K rows zero).
    dt_ = dic.transpose(1, 0, 2, 3)                       # [cin, d, kh, kw]
    wstk = np.zeros((128, 6 * DP), np.float32)
    for kh in range(3):
        wstk[0:64, kh * DP:kh * DP + D] = dt_[:, :, kh, 0]
        wstk[64:128, kh * DP:kh * DP + D] = dt_[:, :, kh, 1]
    wstk[0:64, 3 * DP:3 * DP + D] = dt_[:, :, 1, 2]
    wstk[64:128, 3 * DP:3 * DP + D] = dt_[:, :, 2, 2]
    wstk[0:64, 4 * DP:4 * DP + D] = dt_[:, :, 0, 2]      # (kh0,kw2) rows 0:64
    wstk[64:128, 5 * DP:5 * DP + D] = dt_[:, :, 0, 2]    # (kh0,kw2) rows 64:128
    wstk = wstk.astype(ml_dtypes.bfloat16)

    xpad = np.zeros((B, CIN, PH, PW), np.float32)
    xpad[:, :, 1:H + 1, 1:W + 1] = x
    xpad = xpad.astype(ml_dtypes.bfloat16)

    in_maps = []
    for c in range(NCORES):
        xf = xpad[c * BPC:(c + 1) * BPC].transpose(1, 0, 2, 3).reshape(CIN, F)
        xpk = np.zeros((128, F), ml_dtypes.bfloat16)
        xck = np.zeros((128, F), ml_dtypes.bfloat16)
        xpk[0:64] = xf
        xpk[64:128, 0:F - 1] = xf[:, 1:]
        xck[0:64, 0:F - PW] = xf[:, PW:]
        xck[64:128, 0:F - 2 * PW] = xf[:, 2 * PW:]
        in_maps.append({
            "xp": np.ascontiguousarray(xpk),
            "xc": np.ascontiguousarray(xck),
            "wst": wstk, "wm": wmp,
        })
    return in_maps


def _run(in_maps, trace=False, **kw):
    nc = _get_nc()
    return run_bass_kernel_spmd(nc, in_maps, core_ids=list(range(NCORES)),
                                trace=trace, **kw)


def kernel(x, dictionary, lookup_coefficients, lookup_indices):
    in_maps = _prep_inputs(x, dictionary, lookup_coefficients, lookup_indices)
    res = _run(in_maps)
    outs = [np.asarray(res.results[c]["out"]).astype(np.float32)
            for c in range(NCORES)]
    return np.concatenate(outs, axis=0)
